# revision 1
# baseline (speedup 1.0000x reference)
"""Trainium2 Bass kernel for nn_Column (nms_detection).

Computation (matches the reference exactly):
  out[t,k]  = sum_chw rec_field[t,chw] * weight[k,chw]        (32x512 <- contract 100000)
  pot       = out * (out > 10)
  spike     = (out > 10)
  nspk[k]   = sum_t spike
  first[k]  = min(32 - nspk, 31)
  values[k] = pot[first[k], k]
  v         = max_k(values * (nspk>0)) * 32
  total     = nspk*values + nspk*v
  coef      = top-16 nonzero mask of total (sequential argmax-suppress == top-k set)
  result    = spike * coef[broadcast]                          (32x512 of 0.0/1.0)

Distribution: the contraction dim (100000) is sharded 8 ways (12500 rows/core,
zero-padded to 12544 = 98*128).  Each core computes a partial (32,512) with 98
accumulating PE matmuls (lhsT = X chunk (128,32), rhs = W chunk (128,512)),
then one 64KB AllReduce combines partials; every core redundantly runs the
tiny k-WTA epilogue on-chip and writes the full output (core 0's is returned).

Host-side prep only reshapes/shards/pads inputs and reshapes the output.
"""

import numpy as np

import concourse.bacc as bacc
import concourse.mybir as mybir
from concourse.tile import TileContext
from concourse.bass_utils import run_bass_kernel_spmd

T = 32               # timesteps
K = 512              # out_channels / features
CTOT = 100000        # in_channels * rf_size * length (1*50*2000)
NCORES = 8
SH = CTOT // NCORES  # 12500 contraction rows per core
NCH = 98             # 128-row contraction chunks per core
SHP = NCH * 128      # 12544 (zero padded)
GROUP = 7            # chunks per W DMA group  (7*512*128*4B = 1.75 MiB)
NG = NCH // GROUP    # 14 groups
THRESH = 10.0
F32 = mybir.dt.float32

_CACHE = {}


def _build_nc():
    nc = bacc.Bacc("TRN2", target_bir_lowering=False, debug=False, num_devices=NCORES)

    x_d = nc.dram_tensor("x", [128, NCH * T], F32, kind="ExternalInput")
    w_d = nc.dram_tensor("w", [128, NCH * K], F32, kind="ExternalInput")
    oc_d = nc.dram_tensor("onescol", [T, 1], F32, kind="ExternalInput")
    or_d = nc.dram_tensor("onesrow", [1, T], F32, kind="ExternalInput")
    ti_d = nc.dram_tensor("tiota", [T, 1], F32, kind="ExternalInput")
    out_d = nc.dram_tensor("out", [T, K], F32, kind="ExternalOutput")

    with TileContext(nc) as tc:
        with (
            tc.tile_pool(name="sb", bufs=1) as sb,
            tc.tile_pool(name="wp", bufs=3) as wp,
            tc.tile_pool(name="ps", bufs=1, space="PSUM") as ps,
            tc.tile_pool(name="dram", bufs=1, space="DRAM") as dr,
        ):
            xsb = sb.tile([128, NCH * T], F32)
            nc.sync.dma_start(xsb[:], x_d[:])
            oc = sb.tile([T, 1], F32)
            nc.sync.dma_start(oc[:], oc_d[:])
            orr = sb.tile([1, T], F32)
            nc.sync.dma_start(orr[:], or_d[:])
            ti = sb.tile([T, 1], F32)
            nc.sync.dma_start(ti[:], ti_d[:])

            # ---- matmul: 98 accumulating PE matmuls over the contraction ----
            accum = ps.tile([T, K], F32)
            for g in range(NG):
                wt = wp.tile([128, GROUP * K], F32, tag="wt")
                nc.sync.dma_start(wt[:], w_d[:, g * GROUP * K:(g + 1) * GROUP * K])
                for c in range(GROUP):
                    cc = g * GROUP + c
                    nc.tensor.matmul(
                        accum[:],
                        xsb[:, cc * T:(cc + 1) * T],
                        wt[:, c * K:(c + 1) * K],
                        start=(cc == 0),
                        stop=(cc == NCH - 1),
                    )

            # ---- AllReduce the (32,512) partial across the 8 cores ----
            part = sb.tile([T, K], F32)
            nc.vector.tensor_copy(part[:], accum[:])
            bin_ = dr.tile([T, K], F32)
            bout = dr.tile([T, K], F32)
            nc.gpsimd.dma_start(bin_[:], part[:])
            nc.gpsimd.collective_compute(
                "AllReduce",
                mybir.AluOpType.add,
                replica_groups=[list(range(NCORES))],
                ins=[bin_.opt()],
                outs=[bout.opt()],
            )
            ofull = sb.tile([T, K], F32)
            nc.gpsimd.dma_start(ofull[:], bout[:])

            # ---- threshold fire ----
            spike = sb.tile([T, K], F32)
            nc.vector.tensor_scalar(spike[:], ofull[:], THRESH, None,
                                    op0=mybir.AluOpType.is_gt)
            pot = sb.tile([T, K], F32)
            nc.vector.tensor_tensor(pot[:], spike[:], ofull[:], mybir.AluOpType.mult)

            # nspk[k] = sum_t spike[t,k]   (ones(32,1).T @ spike)
            nspk_ps = ps.tile([1, K], F32)
            nc.tensor.matmul(nspk_ps[:], oc[:], spike[:], start=True, stop=True)
            nspk = sb.tile([1, K], F32)
            nc.vector.tensor_copy(nspk[:], nspk_ps[:])

            # first = min(32 - nspk, 31)
            first = sb.tile([1, K], F32)
            nc.scalar.activation(first[:], nspk[:], mybir.ActivationFunctionType.Copy,
                                 bias=float(T), scale=-1.0)
            nc.vector.tensor_scalar_min(first[:], first[:], float(T - 1))

            # values[k] = pot[first[k], k] via onehot(t == first[k])
            firstb_ps = ps.tile([T, K], F32)
            nc.tensor.matmul(firstb_ps[:], orr[:], first[:], start=True, stop=True)
            onehot = sb.tile([T, K], F32)
            nc.vector.tensor_scalar(onehot[:], firstb_ps[:], ti[:], None,
                                    op0=mybir.AluOpType.is_equal)
            pv = sb.tile([T, K], F32)
            nc.vector.tensor_tensor(pv[:], pot[:], onehot[:], mybir.AluOpType.mult)
            vals_ps = ps.tile([1, K], F32)
            nc.tensor.matmul(vals_ps[:], oc[:], pv[:], start=True, stop=True)
            values = sb.tile([1, K], F32)
            nc.vector.tensor_copy(values[:], vals_ps[:])

            # v = max(values * (nspk > 0)) * T
            sgn = sb.tile([1, K], F32)
            nc.vector.tensor_scalar(sgn[:], nspk[:], 0.0, None,
                                    op0=mybir.AluOpType.is_gt)
            vm = sb.tile([1, K], F32)
            nc.vector.tensor_tensor(vm[:], values[:], sgn[:], mybir.AluOpType.mult)
            vmax = sb.tile([1, 1], F32)
            nc.vector.tensor_reduce(vmax[:], vm[:], axis=mybir.AxisListType.X,
                                    op=mybir.AluOpType.max)
            v_s = sb.tile([1, 1], F32)
            nc.scalar.activation(v_s[:], vmax[:], mybir.ActivationFunctionType.Copy,
                                 bias=0.0, scale=float(T))

            # total = nspk*values + nspk*v
            t1 = sb.tile([1, K], F32)
            nc.vector.tensor_tensor(t1[:], nspk[:], values[:], mybir.AluOpType.mult)
            t2 = sb.tile([1, K], F32)
            nc.vector.tensor_scalar(t2[:], nspk[:], v_s[:], None,
                                    op0=mybir.AluOpType.mult)
            total = sb.tile([1, K], F32)
            nc.vector.tensor_tensor(total[:], t1[:], t2[:], mybir.AluOpType.add)

            # top-16 nonzero mask: two rounds of (8-max, match-replace-with-0);
            # zapped winners are where total != work afterwards.  Zero entries
            # "win" as no-ops (replaced 0 -> 0) and never enter the mask,
            # matching the reference's invalid-winner (-1) behavior.
            work = sb.tile([1, K], F32)
            s8a = sb.tile([1, 8], F32)
            nc.vector.max(s8a[:], total[:])
            nc.vector.match_replace(work[:], s8a[:], total[:], 0.0)
            s8b = sb.tile([1, 8], F32)
            nc.vector.max(s8b[:], work[:])
            nc.vector.match_replace(work[:], s8b[:], work[:], 0.0)

            coef = sb.tile([1, K], F32)
            nc.vector.tensor_tensor(coef[:], total[:], work[:], mybir.AluOpType.subtract)
            nc.vector.tensor_scalar_min(coef[:], coef[:], 1.0)

            # result = spike * coef[broadcast over t]
            coefb_ps = ps.tile([T, K], F32)
            nc.tensor.matmul(coefb_ps[:], orr[:], coef[:], start=True, stop=True)
            res = sb.tile([T, K], F32)
            nc.vector.tensor_tensor(res[:], spike[:], coefb_ps[:], mybir.AluOpType.mult)
            nc.sync.dma_start(out_d[:], res[:])

    nc.compile()
    return nc


def _get_nc():
    if "nc" not in _CACHE:
        _CACHE["nc"] = _build_nc()
    return _CACHE["nc"]


def _pack_inputs(rec_field, weight):
    X = np.ascontiguousarray(np.asarray(rec_field, dtype=np.float32).reshape(T, CTOT))
    W = np.ascontiguousarray(np.asarray(weight, dtype=np.float32).reshape(K, CTOT))
    oc = np.ones((T, 1), np.float32)
    orr = np.ones((1, T), np.float32)
    ti = np.arange(T, dtype=np.float32).reshape(T, 1)
    in_maps = []
    for i in range(NCORES):
        xp = np.zeros((T, SHP), np.float32)
        xp[:, :SH] = X[:, i * SH:(i + 1) * SH]
        wp = np.zeros((K, SHP), np.float32)
        wp[:, :SH] = W[:, i * SH:(i + 1) * SH]
        # (contract, n) -> chunks (NCH,128,n) -> partition-major (128, NCH*n)
        xpk = np.ascontiguousarray(
            xp.T.reshape(NCH, 128, T).transpose(1, 0, 2).reshape(128, NCH * T))
        wpk = np.ascontiguousarray(
            wp.T.reshape(NCH, 128, K).transpose(1, 0, 2).reshape(128, NCH * K))
        in_maps.append({"x": xpk, "w": wpk, "onescol": oc, "onesrow": orr,
                        "tiota": ti})
    return in_maps


def kernel(rec_field, weight, _trace=False, _trace_kwargs=None):
    nc = _get_nc()
    in_maps = _pack_inputs(rec_field, weight)
    r = run_bass_kernel_spmd(nc, in_maps, list(range(NCORES)), trace=_trace,
                             **(_trace_kwargs or {}))
    _CACHE["last_results"] = r
    out = np.asarray(r.results[0]["out"], dtype=np.float32)
    return out.reshape(T, K, 1, 1)
